# revision 143
# baseline (speedup 1.0000x reference)
"""Trainium2 Bass kernel for nn_MixtureOfTokensLayer.

Math (per sequence position s; B=32 tokens form ONE group of k=32):
  logits = x_s @ controller            (32, 8)
  w      = softmax_k(logits)           (32, 8)
  merged = w.T @ x_s                   (8, 1024)   per-expert token mix
  h      = relu(merged_e @ lin1_e)     (8, 512)
  eo     = h_e @ lin2_e                (8, 1024)
  out_s  = w @ eo                      (32, 1024)

Sharding: data-parallel over S (2048 -> 256 per core, 8 cores), no
collectives. All compute bf16 with fp32 PSUM; output bf16, host upcasts.

Dataflow (single s-chunk of 256; x staged HOST-SIDE as x^T only — the
tok-major copy is rebuilt on-chip by PE transposes, saving 16.8 MB of
HBM traffic per core):
  P1 per 16-s tile (512 tokens, 4 blocks of 128), 1-tile A/B skew so the
  PE never waits on the DVE softmax chain:
    A(t): PE-transpose x^T blocks -> xg blocks (bf16 psum, evac split
      DVE/ACT); logits via narrow matmuls (lhsT = x^T block, rhs = ctl
      chunk, out [tok, 8] accumulated over d-chunks); exp (ACT);
      per-token Z via block-diag-ones matmul; then DVE: 1/Z, normalized
      weights, block-diag wbd.
    B(t-1): merged^T directly (lhsT = xg block chunk, rhs = wbd, out
      [d-chunk, 32] — no merge transposes); wbdT via PE transpose.
  P2 per expert: lin1 (w1 chunk stationary, stream merged^T, N=256),
    relu evac -> h (f-major).
  P3: lin2 per (d-half, expert, s-half) (w2 stationary, stream h,
    N=128, the four dt accumulation groups sharing one psum bank) ->
    eoT (d-major); emit runs in skewed pair-of-tile batches
    (PE-transpose eoT -> (s,e)-major blocks, emit matmuls with
    lhsT = wbdT row blocks, psum spread over three pools, evacs split
    DVE/ACT), interleaved between the remaining lin2 sections so the
    first output DMA starts one s-half after lin1 finishes.

Weight DMAs are held behind the latency-critical x stream with
scheduler timestamps (tile_wait_until) on the sync (SP) ring; pool
slot rotation throttles them to just-in-time.
"""

import sys

import numpy as np
import ml_dtypes

sys.path.insert(0, "/opt/trn_rl_repo")

import concourse.bass as bass
import concourse.mybir as mybir
import concourse.tile as tile
from concourse import bacc

B, S, D, F, E, K = 32, 2048, 1024, 512, 8, 32
N_CORES = 8
TILE_S = 16              # s-positions per P1 tile
NBLK = 4                 # 128-token blocks per tile
TOK = 128                # tokens per block (4 s * 32 k)
DC = D // 128            # 8 d-chunks
FC = F // 128            # 4 f-chunks
BF = mybir.dt.bfloat16
F32 = mybir.dt.float32
AF = mybir.ActivationFunctionType


class _Ctx:
    pass


def moe_body(tc, xT, w1, w2, ctl, idb, abd, msk, out, s_local, reps=1):
    nc = tc.nc
    import contextlib

    n_tiles = s_local // TILE_S
    cs = s_local

    with (
        tc.tile_pool(name="const", bufs=1) as const,
        tc.tile_pool(name="pers", bufs=1) as pers,
        tc.tile_pool(name="xstream", bufs=3) as xsp,
        tc.tile_pool(name="xgp", bufs=3) as xgp,
        tc.tile_pool(name="route", bufs=2) as rte,
        tc.tile_pool(name="wstream", bufs=2) as wsp,
        tc.tile_pool(name="outst", bufs=4) as outst,
        tc.tile_pool(name="psA", bufs=3, space="PSUM") as psA,
        tc.tile_pool(name="psB", bufs=2, space="PSUM") as psB,
        tc.tile_pool(name="psC", bufs=3, space="PSUM") as psC,
    ):
        c = _Ctx()
        c.tc, c.nc = tc, nc
        c.xsp, c.xgp, c.rte, c.wsp, c.outst = xsp, xgp, rte, wsp, outst
        c.psA, c.psB, c.psC = psA, psB, psC
        c.w1, c.w2 = w1, w2
        c.n_tiles, c.cs = n_tiles, cs

        # ---- constants ----
        c.idb_sb = const.tile([128, 128], BF, name="idb_sb")
        nc.sync.dma_start(c.idb_sb[:], idb)
        c.ctl_sb = const.tile([128, DC, E], BF, name="ctl_sb")
        nc.scalar.dma_start(c.ctl_sb[:], ctl)
        c.abd_sb = const.tile([128, 128], BF, name="abd_sb")
        nc.scalar.dma_start(c.abd_sb[:], abd)
        c.msk_sb = const.tile([128, NBLK, E], BF, name="msk_sb")
        nc.scalar.dma_start(c.msk_sb[:], msk)

        # ---- persistent intermediates ----
        # merged^T, (s,e)-interleaved columns: [d-part, dc, s, e].
        # eoT ([d-part, dt, s, e]) aliases the same slot later: mT is dead
        # once lin1 has consumed it, so tag-generation rotation reuses it.
        c.pers = pers
        c.mT = pers.tile([128, DC, cs, E], BF, tag="big", bufs=1, name="mT")
        # h, f-major: [f-part, fc, s, e]
        c.h_all = pers.tile([128, FC, cs, E], BF, name="h_all")
        # emit stationary: [(s,e)-part, tile, tok]
        c.wbdT = pers.tile([128, n_tiles, TOK], BF, name="wbdT")

        c.xT_r = xT.rearrange("(dc p) t -> p dc t", p=128)
        c.out_r = out.rearrange("(nb p) d -> p nb d", p=128)

        # w1/w2 stream tiles, DMA'd from the sync ring interleaved with x
        c.w1t = [None] * E
        c.w2t = [None] * (2 * E)

        W1_AT = [46 + 2.9 * i for i in range(E)]
        W2_AT = [65 + 1.5 * i for i in range(2 * E)]

        def load_w1(e, eng=None):
            # hold weight transfers behind the latency-critical x stream
            with tc.tile_wait_until(W1_AT[e] / 1000):
                c.w1t[e] = c.wsp.tile([128, DC, F], BF, tag="w1",
                                      bufs=6, name=f"w1_{e}")
                (eng or nc.sync).dma_start(c.w1t[e][:], c.w1[:, e])

        def load_w2(i):
            half, we = i // E, i % E
            with tc.tile_wait_until(W2_AT[i] / 1000):
                c.w2t[i] = c.wsp.tile([128, 4, FC, 128], BF, tag="w2",
                                      bufs=8, name=f"w2_{half}_{we}")
                nc.sync.dma_start(c.w2t[i][:],
                                  c.w2[:, we, half * 4:(half + 1) * 4])

        rep_ctx = tc.For_i(0, reps, 1) if reps > 1 else contextlib.nullcontext()
        with rep_ctx:
            prev = None  # A-phase state for the 1-tile skew
            for t in range(n_tiles):
                stt = _p1_partA(c, t)
                if prev is not None:
                    _p1_partB(c, prev)
                prev = stt
                # first w1 load rides the very tail of the x stream
                if t == 15:
                    load_w1(0, eng=nc.sync)
            _p1_partB(c, prev)
            # rest of the weights queue behind x on the idle sync ring;
            # pool slot rotation throttles them to just-in-time
            for e in range(1, E):
                load_w1(e)
            for i in range(2 * E):
                load_w2(i)
            for e in range(E):
                _p2_expert(c, e)
            # expert outs, d-major: [d-part, dt, s, e] — reuses mT's slot
            c.eoT = c.pers.tile([128, DC, cs, E], BF, tag="big", bufs=1,
                                name="eoT")
            # lin2 runs in s-halves (s-outer) so each emit batch of 8
            # tiles unblocks after only half the lin2 work of its d-half
            for e in range(E):
                _p3_lin2(c, 0, e, 0)
            gens = [_emit_gen(c, 0, 0, 8), _emit_gen(c, 0, 8, 16),
                    _emit_gen(c, 1, 0, 8)]
            sections = [(0, 1), (1, 0), (1, 1)]
            for g, (half, sh) in zip(gens, sections):
                for e in range(E):
                    _p3_lin2(c, half, e, sh)
                    if e % 2 == 0:
                        next(g, None)
            for g in gens:
                for _ in g:
                    pass
            for _ in _emit_gen(c, 1, 8, 16):
                pass


def _p1_partA(c, t):
    nc = c.nc
    xt = c.xsp.tile([128, DC, TILE_S * K], BF, tag="xT", name=f"xT{t}")
    if t == 0:
        # first tile in two half-loads so the PE starts ~2us earlier
        xh = c.rte.tile([128, DC, 256], BF, tag="x0a", bufs=1, name="x0a")
        nc.sync.dma_start(xh[:], c.xT_r[:, :, 0:256])
        nc.sync.dma_start(xt[:, :, 256:512], c.xT_r[:, :, 256:512])
    else:
        xh = None
        nc.sync.dma_start(xt[:], c.xT_r[:, :, t * 512:(t + 1) * 512])

    xg = c.xgp.tile([128, NBLK, D], BF, tag="xg", name=f"xg{t}")
    expl = c.rte.tile([128, NBLK, E], BF, tag="expl", name=f"expl{t}")
    # one logits psum tile per 16-s tile: the four per-block accumulation
    # groups run sequentially in the same bank
    lg_ps = c.psB.tile([128, NBLK, E], F32, tag="B", name=f"lgps{t}")
    for b in range(NBLK):
        if xh is not None and b < 2:
            blk = xh[:, :, b * TOK:(b + 1) * TOK]
        else:
            blk = xt[:, :, b * TOK:(b + 1) * TOK]
        # transposes: x^T block -> xg block (bf16 psum)
        xg_ps = c.psA.tile([128, DC, TOK], BF, tag="A", name=f"xgps{t}_{b}")
        for dc in range(DC):
            nc.tensor.transpose(xg_ps[:, dc, :], blk[:, dc, :], c.idb_sb[:])
        # logits: [tok, 8] accumulated over d-chunks
        for dc in range(DC):
            nc.tensor.matmul(lg_ps[:, b, :], blk[:, dc, :],
                             c.ctl_sb[:, dc, :],
                             start=(dc == 0), stop=(dc == DC - 1))
        if b % 4 == 0:
            ev = _act_copy(nc)
        else:
            ev = nc.vector.tensor_copy
        ev(out=xg[:, b, :], in_=xg_ps[:].rearrange("p dc k -> p (dc k)"))
    nc.scalar.activation(expl[:], lg_ps[:], AF.Exp)

    # per-token Z (replicated over the 32 tokens of each s)
    z_ps = c.psB.tile([128, NBLK, E], F32, tag="B", name=f"zps{t}")
    for b in range(NBLK):
        nc.tensor.matmul(z_ps[:, b, :], c.abd_sb[:], expl[:, b, :],
                         start=True, stop=True)
    zr = c.rte.tile([128, NBLK, E], F32, tag="zr", name=f"zr{t}")
    nc.vector.reciprocal(zr[:], z_ps[:])
    expn = c.rte.tile([128, NBLK, E], BF, tag="expn", name=f"expn{t}")
    nc.gpsimd.tensor_tensor(out=expn[:], in0=expl[:], in1=zr[:],
                            op=mybir.AluOpType.mult)
    wbd = c.rte.tile([TOK, NBLK, NBLK * E], BF, tag="wbd", name=f"wbd{t}")
    for b in range(NBLK):
        nc.gpsimd.tensor_tensor(
            out=wbd[:, b, :],
            in0=expn[:, b, None, :].to_broadcast((TOK, NBLK, E)),
            in1=c.msk_sb[:],
            op=mybir.AluOpType.mult)

    stt = _Ctx()
    stt.t, stt.xg, stt.wbd = t, xg, wbd
    return stt


def _p1_partB(c, stt):
    nc = c.nc
    t, xg, wbd = stt.t, stt.xg, stt.wbd
    wt_ps = c.psC.tile([128, TOK], BF, tag="Cmt", name=f"wtps{t}")
    for p in range(2):  # block pairs share a 2-bank psum tile
        mt_ps = c.psC.tile([128, 2, DC, NBLK * E], F32, tag="Cmt",
                           name=f"mtps{t}_{p}")
        for w in range(2):
            b = p * 2 + w
            # merged^T: lhsT = xg block chunk, rhs = wbd -> [d-chunk, 32]
            for dc in range(DC):
                nc.tensor.matmul(mt_ps[:, w, dc, :],
                                 xg[:, b, dc * 128:(dc + 1) * 128],
                                 wbd[:, b, :], start=True, stop=True)
            r0 = 32 * b
            nc.tensor.transpose(wt_ps[r0:r0 + 32, :], wbd[:, b, :],
                                c.idb_sb[:], tile_position=(0, r0))
        s0 = t * TILE_S + p * 2 * NBLK
        dst = c.mT[:, :, s0:s0 + 2 * NBLK, :]
        # last tile: split across engines so lin1 unblocks sooner
        if t == c.n_tiles - 1 and p == 1:
            ev = nc.vector.tensor_copy
        else:
            ev = _act_copy(nc)
        ev(out=dst.rearrange("p dc (w s) e -> p w dc s e", w=2),
           in_=mt_ps[:].rearrange("p w dc (s e) -> p w dc s e", e=E))
    nc.vector.tensor_copy(out=c.wbdT[:, t, :], in_=wt_ps[:])


def _act_copy(nc):
    def f(out, in_):
        return nc.scalar.copy(out=out, in_=in_)
    return f


def _p2_expert(c, e):
    nc = c.nc
    w1t = c.w1t[e]
    for fp in range(2):  # ft pairs
        h_ps = c.psA.tile([128, 2, c.cs], F32, tag="A", name=f"hps{e}_{fp}")
        for j in range(2):
            ft = fp * 2 + j
            for dc in range(DC):
                nc.tensor.matmul(h_ps[:, j, :],
                                 w1t[:, dc, ft * 128:(ft + 1) * 128],
                                 c.mT[:, dc, :, e],
                                 start=(dc == 0), stop=(dc == DC - 1))
        nc.scalar.activation(c.h_all[:, fp * 2:(fp + 1) * 2, :, e], h_ps[:],
                             AF.Relu)


def _p3_lin2(c, half, e, sh, nq=2):
    nc = c.nc
    w2t = c.w2t[half * E + e]
    hs = c.cs // nq
    s0 = sh * hs
    # all four dt accumulation groups share one psum bank (sequential)
    eo_ps = c.psA.tile([128, 4, hs], F32, tag="A",
                       name=f"eops{half}_{e}_{sh}")
    for dt in range(4):
        for fc in range(FC):
            nc.tensor.matmul(eo_ps[:, dt, :],
                             w2t[:, dt, fc, :],
                             c.h_all[:, fc, s0:s0 + hs, e],
                             start=(fc == 0), stop=(fc == FC - 1))
    dst = c.eoT[:, half * 4:(half + 1) * 4, s0:s0 + hs, e]
    nc.vector.tensor_copy(out=dst, in_=eo_ps[:])


def _emit_gen(c, half, t0, t1):
    """Pair-of-tiles emit, skewed: the next pair's PE transposes issue
    before the current pair's emit matmuls/evacs; the next pair's eo_blk
    copy issues after them so the in-order DVE queue drains the current
    evacs first."""
    prev = None
    for t in range(t0, t1, 2):
        a = _p3_emitA1(c, half, t)
        if prev is not None:
            _p3_emitB(c, prev)
        _p3_emitA2(c, a)
        if prev is not None:
            yield
        prev = a
    _p3_emitB(c, prev)
    yield


def _p3_emitA1(c, half, t):
    nc = c.nc
    # transpose two eoT tiles -> (s,e)-major blocks
    eb_ps = c.psC.tile([128, 2, 4, 128], BF, tag="Cmt",
                       name=f"ebps{half}_{t}")
    for w in range(2):
        for j in range(4):
            dt = half * 4 + j
            src = c.eoT[:, dt, (t + w) * TILE_S:(t + w + 1) * TILE_S, :]
            nc.tensor.transpose(eb_ps[:, w, j, :],
                                src.rearrange("p s e -> p (s e)"),
                                c.idb_sb[:])
    stt = _Ctx()
    stt.half, stt.t, stt.eb_ps = half, t, eb_ps
    return stt


def _p3_emitA2(c, stt):
    nc = c.nc
    eo_blk = c.rte.tile([128, 2, 4, 128], BF, tag="eoblk",
                        name=f"eob{stt.half}_{stt.t}")
    ev = nc.vector.tensor_copy if stt.t % 4 else _act_copy(nc)
    ev(out=eo_blk[:], in_=stt.eb_ps[:])
    stt.eo_blk = eo_blk


def _p3_emitB(c, stt):
    nc = c.nc
    half, t, eo_blk = stt.half, stt.t, stt.eo_blk
    for w in range(2):
        o_sb = c.outst.tile([128, NBLK, 512], BF, tag="osb",
                            name=f"osb{half}_{t}_{w}")
        for b in range(NBLK):
            r0 = 32 * b
            pool, tag = [(c.psB, "B"), (c.psC, "Cmt"), (c.psA, "A"),
                         (c.psC, "Cmt")][b]
            o_ps = pool.tile([128, 512], F32, tag=tag,
                             name=f"ops{half}_{t}_{w}_{b}")
            nc.tensor.matmul(o_ps[:], c.wbdT[r0:r0 + 32, t + w, :],
                             eo_blk[r0:r0 + 32, w, :, :],
                             start=True, stop=True, tile_position=(r0, 0))
            ev = _act_copy(nc) if b % 2 else nc.vector.tensor_copy
            ev(out=o_sb[:, b, :], in_=o_ps[:])
        if half == 1 and t + w == 15:
            # final tile: two half-DMAs so the last transfer is shorter
            for hh in range(2):
                nc.scalar.dma_start(
                    c.out_r[:, (t + w) * NBLK + hh * 2:
                            (t + w) * NBLK + (hh + 1) * 2,
                            half * 512:(half + 1) * 512],
                    o_sb[:, hh * 2:(hh + 1) * 2, :])
        else:
            nc.scalar.dma_start(
                c.out_r[:, (t + w) * NBLK:(t + w + 1) * NBLK,
                        half * 512:(half + 1) * 512],
                o_sb[:])


def build_module(s_local, num_devices, reps=1):
    T = s_local * K
    nc = bacc.Bacc("TRN2", target_bir_lowering=False, debug=False,
                   num_devices=num_devices)
    xT = nc.dram_tensor("xT", [D, T], BF, kind="ExternalInput").ap()
    w1 = nc.dram_tensor("w1", [128, E, DC, F], BF, kind="ExternalInput").ap()
    w2 = nc.dram_tensor("w2", [128, E, DC, FC, 128], BF,
                        kind="ExternalInput").ap()
    ctl = nc.dram_tensor("ctl", [128, DC, E], BF, kind="ExternalInput").ap()
    idb = nc.dram_tensor("idb", [128, 128], BF, kind="ExternalInput").ap()
    abd = nc.dram_tensor("abd", [128, 128], BF, kind="ExternalInput").ap()
    msk = nc.dram_tensor("msk", [128, NBLK, E], BF, kind="ExternalInput").ap()
    out = nc.dram_tensor("out", [T, D], BF, kind="ExternalOutput").ap()
    with tile.TileContext(nc) as tc:
        moe_body(tc, xT, w1, w2, ctl, idb, abd, msk, out, s_local, reps=reps)
    nc.compile()
    return nc


def stage_weights(lin1, lin2, controller):
    bf = ml_dtypes.bfloat16
    # [128p, e, dc, f]: element = lin1[e, dc*128+p, f]
    w1h = np.ascontiguousarray(
        lin1.reshape(E, DC, 128, F).transpose(2, 0, 1, 3)).astype(bf)
    # [128p, e, dt, fc, c]: element = lin2[e, fc*128+p, dt*128+c]
    w2h = np.ascontiguousarray(
        lin2.reshape(E, FC, 128, DC, 128).transpose(2, 0, 3, 1, 4)).astype(bf)
    ctlh = np.ascontiguousarray(
        controller.reshape(DC, 128, E).transpose(1, 0, 2)).astype(bf)
    return w1h, w2h, ctlh


def stage_consts():
    bf = ml_dtypes.bfloat16
    idb = np.eye(128, dtype=np.float32).astype(bf)
    # block-diag ones: A[i, j] = 1 iff same s-position (i//32 == j//32)
    abd = np.kron(np.eye(NBLK, dtype=np.float32),
                  np.ones((K, K), np.float32)).astype(bf)
    msk = np.zeros((128, NBLK, E), np.float32)
    for sb in range(NBLK):
        msk[sb * K:(sb + 1) * K, sb, :] = 1.0
    return idb, abd, msk.astype(bf)


def stage_x(xs):
    """xs: (B, s_local, D) fp32 -> xT bf16 (D, T), d-major."""
    s_local = xs.shape[1]
    bf = ml_dtypes.bfloat16
    xT_h = np.ascontiguousarray(
        xs.transpose(2, 1, 0).reshape(D, s_local * K)).astype(bf)
    return xT_h


_MODULE_CACHE = {}


def kernel(x, lin1, lin2, controller):
    from concourse.bass_utils import run_bass_kernel_spmd

    s_local = S // N_CORES
    key = (s_local, N_CORES)
    if key not in _MODULE_CACHE:
        _MODULE_CACHE[key] = build_module(s_local, N_CORES)
    nc = _MODULE_CACHE[key]

    w1h, w2h, ctlh = stage_weights(lin1, lin2, controller)
    idb, abd, msk = stage_consts()
    in_maps = []
    for c in range(N_CORES):
        xT_h = stage_x(x[:, c * s_local:(c + 1) * s_local, :])
        in_maps.append({"xT": xT_h, "w1": w1h, "w2": w2h,
                        "ctl": ctlh, "idb": idb, "abd": abd, "msk": msk})

    res = run_bass_kernel_spmd(nc, in_maps, core_ids=list(range(N_CORES)))
    out_full = np.empty((B, S, D), np.float32)
    for c in range(N_CORES):
        oc = np.asarray(res.results[c]["out"]).astype(np.float32)
        out_full[:, c * s_local:(c + 1) * s_local, :] = (
            oc.reshape(s_local, K, D).transpose(1, 0, 2))
    kernel.last_results = res
    return out_full
